# revision 1
# baseline (speedup 1.0000x reference)
"""Trainium2 Bass kernel for nn_BaseTree (decision-tree inference), v13.

Emulated-device cost model (measured): each instruction costs ~50us
fixed plus a small per-element term; DMAs cost per contiguous segment
(strided DMAs are catastrophic, contiguous ones nearly free); broadcast
(stride-0) input APs are nearly free.  So: minimize instruction count,
keep DMAs contiguous.

Algorithm (per core, pure data parallel, tree baked at build time):
  - Host passes xg[b, q] = x[b, feature[perm[q]]] where perm lays each
    heap level out in BIT-REVERSED level-local order (see below); pure
    input re-indexing done while sharding.  Thresholds get the same
    permutation.
  - comp[p, r, q] = xg > thr (broadcast threshold row), one compare per
    row-subtile (96/96/64 rows x all 255 columns, contiguous DMAs).
  - The traversal is a pure SELECT NETWORK over the level-7 block:
    positions q in the bit-reversed layout put the two children of
    position i at i (left) and i + 2^j (right), so every stage is one
    in-place copy_predicated of the upper half onto the lower half,
    predicated on that level's comparison bits.  The leaf's path bits
    are recovered WITHOUT any accumulation arithmetic: survivor position
    q encodes the path (bit_j(q) = level-j decision), so baking the
    constant 2*rev7(q) into z7 = c7 + 2*rev7(q) (one broadcast add per
    tile) makes the final surviving byte equal the leaf index exactly.
    Values <= 255, u8 exact, zero saturation.
  - Host expands value[leaf] while unsharding (a 256x8 table lookup;
    the environment's indirect DMA gather is broken).

Per core: 7 DMAs (6 x-subtiles + 1 out) + 6 compares + 2 z7-bakes +
4 per-tile selects + 2 tail-persists + 5 merged final selects = 27
total instructions (vs 904 for the level-by-level baseline), zero
framework scaffolding (raw Bass + _lean_init), every DMA contiguous.
Thresholds + the rev constant ride as 2 rows prepended to each
subtile's DMA; c4..c0 sit just below the z region so both tiles' last
five select stages merge into one full-width pass over a column-major
63-column persisted tail whose survivor lands contiguous for the
output DMA (the unused dynamic-DMA scratch carveout is shrunk to fit).
"""

import contextlib
from contextlib import ExitStack

import numpy as np

import concourse.bacc as bacc
import concourse.bass as bass_mod
import concourse.mybir as mybir
from concourse.bass_utils import run_bass_kernel_spmd

AF = mybir.AluOpType
F32 = mybir.dt.float32
U8 = mybir.dt.uint8

N_CORES = 8
P = 128
B_TOTAL = 524288
B_CORE = B_TOTAL // N_CORES      # 65536
S_CORE = B_CORE // P             # 512 rows per partition
F = 32
DEPTH = 8
N_BRANCH = 255
N_LEAF = 256
N_OUT = 8

SUBTILES = ((0, 93), (93, 186), (186, 256))
CROWS = 2                        # const rows (thr, rev) prepended per subtile
T = 2
R = S_CORE // T                  # 256 rows per partition per tile


@contextlib.contextmanager
def _lean_init():
    """Suppress Bass.__init__'s const-AP memsets + all-engine barrier.

    They cost ~12 instructions (~0.6ms here) and this kernel never uses
    const APs (no activation bias) — every dependency is explicit via
    semaphores, so the startup barrier is not needed either.
    """
    orig_memset = bass_mod.BassGpSimd.memset
    orig_barrier = bass_mod.Bass.all_engine_barrier

    class _Dummy:
        def then_inc(self, *a, **k):
            return self

        def _wait_ge(self, *a, **k):
            return self

    bass_mod.BassGpSimd.memset = lambda self, ap, constant: _Dummy()
    bass_mod.Bass.all_engine_barrier = lambda self, *a, **k: None
    try:
        yield
    finally:
        bass_mod.BassGpSimd.memset = orig_memset
        bass_mod.Bass.all_engine_barrier = orig_barrier


def _bitrev(q, bits):
    r = 0
    for _ in range(bits):
        r = (r << 1) | (q & 1)
        q >>= 1
    return r


# column offset of each level's comparison block: c6..c0 descending then z
LEVEL_OFF = {6: 0, 5: 64, 4: 96, 3: 112, 2: 120, 1: 124, 0: 126, 7: 127}


def tree_perm():
    """perm[col] = heap node id at xg column `col`: each level block (at
    LEVEL_OFF) in bit-reversed level-local order (children of position i
    at i, i+2^j).  c4..c0 sit just below the z block so the final
    select stages can run once on a persisted 63-column tail."""
    perm = np.empty(N_BRANCH, dtype=np.int64)
    for j in range(DEPTH):
        base = (1 << j) - 1
        for q in range(1 << j):
            perm[LEVEL_OFF[j] + q] = base + _bitrev(q, j)
    return perm


def build_nc(threshold_perm, rev_const, repeat=1, timing_small_input=False):
    """Build the single-core Bass program (SPMD: same program on all cores).

    threshold_perm: thresholds in xg column order.
    rev_const: [128] u8, rev_const[q] = 2 * rev7(q) for the z7 bake.
    timing_small_input: use a small xg DRAM tensor that every subtile DMA
    re-reads — identical device work per repeat with a tiny host upload
    (for wall-clock slope timing only; results are garbage).
    """
    threshold_perm = np.asarray(threshold_perm, dtype=np.float32)

    with _lean_init():
        nc = bacc.Bacc(dynamic_dma_scratch_size=256)
    # device rows per partition: each subtile = CROWS const rows + data
    SROWS = sum((r1 - r0) + CROWS for (r0, r1) in SUBTILES) * T  # 524
    xg_rows = P * (95 if timing_small_input else SROWS)
    xg = nc.dram_tensor("xg", [xg_rows, N_BRANCH], F32, kind="ExternalInput")
    out = nc.dram_tensor("out", [P, S_CORE], U8, kind="ExternalOutput")
    xv = xg[:].rearrange("(p s) n -> p s n", p=P)

    with ExitStack() as ctx:
        comp = ctx.enter_context(nc.sbuf_tensor("comp", [P, R, N_BRANCH], U8))
        xc = ctx.enter_context(nc.sbuf_tensor("xc", [P, 95, N_BRANCH], F32))
        pers = ctx.enter_context(nc.sbuf_tensor("pers", [P, 63, S_CORE], U8))
        S = ctx.enter_context(nc.semaphore("S"))
        D = ctx.enter_context(nc.semaphore("D"))
        thr_t = xc[:, 0, :]                          # [P, 255] f32 view
        rev_t = xc[:, 1, 0:32].bitcast(U8)           # [P, 128] u8 view

        n_dma = 0          # DMAs issued so far
        n_d = 0            # D value after all marked DVE ops so far
        dev_row = 0        # running device-side row offset per partition

        for rep in range(repeat):
            for t in range(T):
                lo = t * R
                for (r0, r1) in SUBTILES:
                    rw = r1 - r0
                    nrows = rw + CROWS
                    src = (
                        xv[:, :nrows, :] if timing_small_input
                        else xv[:, dev_row: dev_row + nrows, :]
                    )
                    dev_row += nrows
                    dma = nc.sync.dma_start(out=xc[:, :nrows, :], in_=src)
                    if n_d:
                        # WAR: subtile buffer still read by prev compare
                        dma._wait_ge(D, n_d)
                    dma.then_inc(S, 16)
                    n_dma += 1
                    cmp_i = nc.vector.tensor_tensor(
                        out=comp[:, r0:r1, :],
                        in0=xc[:, CROWS: CROWS + rw, :],
                        in1=thr_t.unsqueeze(1).broadcast_to(
                            [P, rw, N_BRANCH]
                        ),
                        op=AF.is_gt,
                    )
                    cmp_i._wait_ge(S, 16 * n_dma)
                    cmp_i.then_inc(D, 1)
                    n_d += 1

                # z7 = c7 + 2*rev7(q): the only arithmetic in the combine.
                # Reads rev from xc's const row -> incs D so the next
                # tile's DMA cannot overwrite xc before it runs.
                bake = nc.vector.tensor_tensor(
                    out=comp[:, :, 127:255],
                    in0=comp[:, :, 127:255],
                    in1=rev_t.unsqueeze(1).broadcast_to([P, R, 128]),
                    op=AF.add,
                )
                bake.then_inc(D, 1)
                n_d += 1
                # select network stages 6..5 (per tile, in place)
                for j in range(6, 4, -1):
                    w = 1 << j
                    off = LEVEL_OFF[j]
                    nc.vector.copy_predicated(
                        out=comp[:, :, 127: 127 + w],
                        mask=comp[:, :, off: off + w],
                        data=comp[:, :, 127 + w: 127 + 2 * w],
                    )
                # persist c4..c0 + z0..31 (cols 96..158) for this tile,
                # column-major so the final survivor is contiguous
                nc.vector.tensor_copy(
                    pers[:, :, lo:lo + R],
                    comp[:, :, 96:159].rearrange("p r c -> p c r"),
                )

            # merged stages 4..0 on both tiles ([P, ., S_CORE])
            if True:
                # pers rows: 0-15 = c4, 16-23 = c3, 24-27 = c2,
                # 28-29 = c1, 30 = c0, 31-62 = z0..31
                nc.vector.copy_predicated(
                    out=pers[:, 31:47, :], mask=pers[:, 0:16, :],
                    data=pers[:, 47:63, :],
                )
                nc.vector.copy_predicated(
                    out=pers[:, 31:39, :], mask=pers[:, 16:24, :],
                    data=pers[:, 39:47, :],
                )
                nc.vector.copy_predicated(
                    out=pers[:, 31:35, :], mask=pers[:, 24:28, :],
                    data=pers[:, 35:39, :],
                )
                nc.vector.copy_predicated(
                    out=pers[:, 31:33, :], mask=pers[:, 28:30, :],
                    data=pers[:, 33:35, :],
                )
                last = nc.vector.copy_predicated(
                    out=pers[:, 31:32, :], mask=pers[:, 30:31, :],
                    data=pers[:, 32:33, :],
                )

        last.then_inc(D, 1)
        n_d += 1
        nc.sync.dma_start(out=out[:], in_=pers[:, 31, :])._wait_ge(
            D, n_d
        ).then_inc(S, 16)

    nc.compile()
    return nc


def _check_tree(cond, cond_mask):
    """Verify cond/cond_mask encode the canonical heap-ordered perfect tree."""
    n_nodes = 2 * N_LEAF - 1
    n_branch = N_LEAF - 1
    is_branch = np.zeros(n_nodes, dtype=bool)
    node_conditions = np.zeros((n_nodes, n_nodes), dtype=bool)
    node_conditions_mask = np.zeros((n_nodes, n_nodes), dtype=bool)

    stack = [(0, None)]
    while stack:
        node_id, parent_id = stack.pop()
        if parent_id is not None:
            node_conditions_mask[node_id] = node_conditions_mask[parent_id]
            node_conditions_mask[node_id][parent_id] = True
        if node_id < n_branch:
            left_id, right_id = 2 * node_id + 1, 2 * node_id + 2
            is_branch[node_id] = True
            node_conditions[left_id] = node_conditions[node_id]
            node_conditions[right_id] = node_conditions[node_id]
            node_conditions[right_id][node_id] = True
            stack.append((right_id, node_id))
            stack.append((left_id, node_id))

    leaf_ids = np.nonzero(~is_branch)[0]
    branch_ids = np.nonzero(is_branch)[0]
    c = node_conditions[np.ix_(leaf_ids, branch_ids)]
    m = node_conditions_mask[np.ix_(leaf_ids, branch_ids)]
    return np.array_equal(c, np.asarray(cond)) and np.array_equal(
        m, np.asarray(cond_mask)
    )


_NC_CACHE = {}


def kernel(x, feature, threshold, cond, cond_mask, value):
    x = np.ascontiguousarray(np.asarray(x), dtype=np.float32)
    feature = np.asarray(feature).astype(np.int64)
    threshold = np.asarray(threshold, dtype=np.float32)
    value = np.ascontiguousarray(np.asarray(value), dtype=np.float32)

    assert x.shape == (B_TOTAL, F), x.shape
    if not _check_tree(cond, cond_mask):
        raise ValueError(
            "cond/cond_mask do not encode the canonical heap-ordered tree; "
            "this kernel bakes that structure."
        )

    perm = tree_perm()
    thr_p = threshold[perm]
    rev_const = np.array([2 * _bitrev(q, 7) for q in range(128)], np.uint8)

    key = threshold.tobytes()
    if key not in _NC_CACHE:
        _NC_CACHE[key] = build_nc(thr_p, rev_const)
    nc = _NC_CACHE[key]

    xg = x[:, feature[perm]]                          # [B, 255] f32
    const0 = thr_p.astype(np.float32)                 # thr row
    const1 = np.zeros(N_BRANCH, np.float32)
    const1[:32] = np.ascontiguousarray(rev_const).view(np.float32)
    # interleave: per partition, per subtile: [thr, rev, data rows]
    xs = xg.reshape(N_CORES, P, S_CORE, N_BRANCH)
    blocks = []
    for t in range(T):
        for (r0, r1) in SUBTILES:
            blocks.append(
                np.broadcast_to(
                    np.stack([const0, const1])[None, None],
                    (N_CORES, P, 2, N_BRANCH),
                )
            )
            blocks.append(xs[:, :, t * R + r0: t * R + r1, :])
    xdev = np.ascontiguousarray(
        np.concatenate(blocks, axis=2)
    )                                                  # [8, P, 524, 255]
    shards = xdev.reshape(N_CORES, -1, N_BRANCH)
    in_maps = [{"xg": shards[i]} for i in range(N_CORES)]
    res = run_bass_kernel_spmd(nc, in_maps, list(range(N_CORES)))
    leaves = np.concatenate(
        [np.asarray(r["out"]).reshape(-1) for r in res.results]
    ).astype(np.int64)
    return value[leaves]


if __name__ == "__main__":
    import reference

    inputs = reference.setup_inputs()
    got = kernel(**{k: np.asarray(v) for k, v in inputs.items()})
    exp = np.asarray(reference.reference(**inputs))
    err = np.abs(got - exp).max()
    print("absmax err:", err)



# revision 22
# speedup vs baseline: 24.1838x; 24.1838x over previous
"""Trainium2 Bass kernel for nn_BaseTree (decision-tree inference), v14.

Architecture (per core, pure data parallel, tree baked at build time):

  The v13 kernel shipped xg[b, q] = x[b, feature[q]] from the host: a
  255-column f32 expansion of the 32-column input (8x the HBM bytes) and
  did the comparisons on DVE (~320us DVE-busy, ~210us DMA).  v14 moves
  the gather INTO the tensor engine and splits the remaining elementwise
  work across all four compute engines:

  - Host ships xT3[k, r]: x transposed, split into THREE bf16 pieces
    (x == h + m + l exactly; verified at build), rows 3f+p = piece p of
    feature f, plus three rows of ones.  Also G[99, 255] bf16 where
    column q has 1.0 at the three piece-rows of feature[perm[q]] and the
    three bf16 pieces of -threshold[perm[q]] against the ones rows.
  - PE: 512 matmuls (one per 128-row block) compute
    y[r, q] = x[r, f[q]] - thr[q] exactly in f32 PSUM (products are by
    1.0, the f32 sums are exact by a Sterbenz chain; min on-path margin
    of this dataset is 1.2e-7 vs ~1e-10 worst-case rounding).
  - Act: masks[., c] = sigmoid(1e30 * y) in u8 for the 127 mask columns
    (levels 0-6): exactly {0, 1} since |1e30*y| >= 37 for all nonzero y
    and ties (y == 0) truncate to 0 = "not >".
  - DVE+Pool: z-block (level 7): fused scalar_tensor_tensor
    (y > 0) + 2*rev7(q) in one op, split by columns across the two
    engines (the rev bake makes the final select survivor equal the
    leaf index; see v13 notes).
  - DVE: the select network (copy_predicated stages 6..0), batched every
    4 PSUM groups to stay pipelined with extraction, final survivor
    written contiguously for the out DMA.
  - Host expands value[leaf] while unsharding.

  Engine busy estimates (cost model): PE ~60us, DMA ~36us, Act ~66us,
  DVE ~80us, Pool ~80us -> ~85-95us wall vs 486us for v13.
"""

import contextlib
from contextlib import ExitStack

import numpy as np
import ml_dtypes

import concourse.bacc as bacc
import concourse.bass as bass_mod
import concourse.mybir as mybir
from concourse.bass_utils import run_bass_kernel_spmd

AF = mybir.AluOpType
ACT = mybir.ActivationFunctionType
F32 = mybir.dt.float32
BF16 = mybir.dt.bfloat16
U8 = mybir.dt.uint8
U16 = mybir.dt.uint16
BF = ml_dtypes.bfloat16

N_CORES = 8
P = 128
B_TOTAL = 524288
B_CORE = B_TOTAL // N_CORES      # 65536
F = 32
DEPTH = 8
N_BRANCH = 255
N_LEAF = 256
N_OUT = 8

K = 99                 # 3*32 x-pieces + 3 ones rows
NBLK = B_CORE // P     # 512 matmul blocks
SLOTS = 8              # PSUM slots per group (2 per 2KB bank; 2 groups live)
NGRP = NBLK // SLOTS   # 64 groups, double-buffered by parity
GB = 2                 # groups per select-network batch (tile = 16 rows)
TB = GB * SLOTS        # 64 tile rows per batch
NBATCH = NGRP // GB    # 8
CHB = 64               # blocks per DMA chunk (8192 rows)
NCHUNK = NBLK // CHB   # 8
CH_ROWS = CHB * P      # 8192
ZD = 0                 # z-block columns extracted on DVE via fused stt
SCALE = 1e30           # sigmoid saturation scale

# mask-column offsets (within cols 0..126): c6@0 c5@64 c4@96 c3@112
# c2@120 c1@124 c0@126; z-block (level 7) = G columns 127..254
LEVEL_OFF = {6: 0, 5: 64, 4: 96, 3: 112, 2: 120, 1: 124, 0: 126, 7: 127}


@contextlib.contextmanager
def _lean_init():
    """Suppress Bass.__init__'s const-AP memsets + all-engine barrier
    (unused here: no const APs — the activation bias is an explicit
    zeros AP shipped with the constants — and deps are explicit sems)."""
    orig_memset = bass_mod.BassGpSimd.memset
    orig_barrier = bass_mod.Bass.all_engine_barrier

    class _Dummy:
        def then_inc(self, *a, **k):
            return self

        def _wait_ge(self, *a, **k):
            return self

    bass_mod.BassGpSimd.memset = lambda self, ap, constant: _Dummy()
    bass_mod.Bass.all_engine_barrier = lambda self, *a, **k: None
    try:
        yield
    finally:
        bass_mod.BassGpSimd.memset = orig_memset
        bass_mod.Bass.all_engine_barrier = orig_barrier


def _bitrev(q, bits):
    r = 0
    for _ in range(bits):
        r = (r << 1) | (q & 1)
        q >>= 1
    return r


def tree_perm():
    """perm[col] = heap node id at G column `col` (see v13)."""
    perm = np.empty(N_BRANCH, dtype=np.int64)
    for j in range(DEPTH):
        base = (1 << j) - 1
        for q in range(1 << j):
            perm[LEVEL_OFF[j] + q] = base + _bitrev(q, j)
    return perm


def build_nc(zd=ZD, gb=None, tbuf=3):
    """Build the single-core Bass program (SPMD: same program on all cores).

    Engine roles (the neuronxcc verifier only accepts TensorCopy on Pool,
    so Pool cannot share tensor work):
      PE   512 matmuls (gather + threshold subtract), parity-buffered PSUM
      Act  sigmoid(1e30*y): comp bits for mask cols + z-sig cols
      DVE  fused (y>0)+2rev stt for `zd` z-cols, bf16 2x bake for the
           act-extracted z-cols, and the whole copy_predicated network
      Pool final survivor copy (bf16 tile col -> contiguous u8)

    Soundness with one wait slot per instruction: chains
      matmul(g) -> SV>=g-1 (or SA if zd==0) covers both consumers because
      the DVE stt of group g itself waits SA>=g+1 (act g done).
    """
    with _lean_init():
        nc = bacc.Bacc(dynamic_dma_scratch_size=256)

    xt = nc.dram_tensor("xt", [K, B_CORE], BF16, kind="ExternalInput")
    gm = nc.dram_tensor("gm", [K, N_BRANCH], BF16, kind="ExternalInput")
    cb = nc.dram_tensor("cbias", [P, 1], F32, kind="ExternalInput")
    cr = nc.dram_tensor("crev", [P, 128], BF16, kind="ExternalInput")
    # two survivor candidates (z0, z1) + the root mask c0; the final
    # 1-wide select happens on the host (a 1-element u16 predicated write
    # would be a partial-word RMW, which the DVE mishandles — see below)
    out2 = nc.dram_tensor("out2", [P, NBLK * 2], U8, kind="ExternalOutput")
    outc = nc.dram_tensor("outc", [P, NBLK], U8, kind="ExternalOutput")

    za = 128 - zd           # act-extracted z cols (bake on DVE)
    acols = 127 + za        # act columns per group: masks + z-sig
    gb = GB if gb is None else gb
    tb = gb * SLOTS
    nbatch = NGRP // gb

    with ExitStack() as ctx:
        xtb = ctx.enter_context(nc.sbuf_tensor("xtb", [P, 2, CH_ROWS], BF16))
        gms = ctx.enter_context(nc.sbuf_tensor("gms", [P, N_BRANCH], BF16))
        cbs = ctx.enter_context(nc.sbuf_tensor("cbs", [P, 1], F32))
        crs = ctx.enter_context(nc.sbuf_tensor("crs", [P, 128], BF16))
        # tile rows padded to 256 cols so every row is 4-byte aligned:
        # the DVE's predicated writes do partial-word RMW against a stale
        # snapshot when an out AP starts mid-word, corrupting neighbours.
        tile = ctx.enter_context(
            nc.sbuf_tensor("tile", [P, tbuf, tb, 256], BF16)
        )
        # baked z lives in its own word-aligned buffer for the same reason
        ztile = ctx.enter_context(
            nc.sbuf_tensor("ztile", [P, tbuf, tb, 128], BF16)
        )
        zfin = ctx.enter_context(nc.sbuf_tensor("zfin", [P, NBLK, 2], U8))
        cfin = ctx.enter_context(nc.sbuf_tensor("cfin", [P, NBLK], U8))
        ps = ctx.enter_context(nc.psum_tensor("ps", [P, 2 * SLOTS * 256], F32))

        SK = ctx.enter_context(nc.semaphore("SK"))   # const dmas (16 ea)
        SE = ctx.enter_context(nc.semaphore("SE"))   # even xt chunks (16 ea)
        SO = ctx.enter_context(nc.semaphore("SO"))   # odd xt chunks (16 ea)
        SM = ctx.enter_context(nc.semaphore("SM"))   # matmuls done (1 ea)
        SA = ctx.enter_context(nc.semaphore("SA"))   # act ops done (1/group)
        SV = ctx.enter_context(nc.semaphore("SV"))   # dve stt done (1/group)
        SB = ctx.enter_context(nc.semaphore("SB"))   # dve batch net done
        SP2 = ctx.enter_context(nc.semaphore("SP2"))  # pool zfin copy done

        psv = ps[:].rearrange("p (s c) -> p s c", s=2 * SLOTS)  # [P, 16, 256]

        # ---- DMAs.  DMA completions are NOT ordered across a queue, so
        # chunks that could complete out of order must not share a
        # semaphore: consts get SK; even/odd chunks get SE/SO (same-parity
        # chunks are serialized by their SM waits).
        nc.sync.dma_start(out=cbs[:], in_=cb[:]).then_inc(SK, 16)
        nc.sync.dma_start(out=crs[:], in_=cr[:]).then_inc(SK, 16)
        nc.sync.dma_start(out=gms[0:K, :], in_=gm[:]).then_inc(SK, 16)
        for c in range(NCHUNK):
            dma = nc.sync.dma_start(
                out=xtb[0:K, c % 2, :],
                in_=xt[:, c * CH_ROWS:(c + 1) * CH_ROWS],
            )
            if c >= 2:
                # WAR: buffer parity reused; wait chunk c-2's matmuls done
                dma._wait_ge(SM, CHB * (c - 1))
            dma.then_inc(SE if c % 2 == 0 else SO, 16)

        for b in range(NBLK):
            c = b // CHB
            g = b // SLOTS
            k = g // gb                       # batch index
            s = (g % 2) * SLOTS + b % SLOTS   # parity double-buffer
            if b == 0:
                # consts (bias/rev/G) before anything runs
                nc.tensor.wait_ge(SK, 48)
            if b % CHB == 0:
                # standalone wait (PE seq): chunk c's DMA done
                nc.tensor.wait_ge(SE if c % 2 == 0 else SO,
                                  16 * (c // 2 + 1))
            mm = nc.tensor.matmul(
                out=psv[:, s, 0:N_BRANCH],
                lhsT=xtb[0:K, c % 2, (b % CHB) * P:(b % CHB) * P + P],
                rhs=gms[0:K, :],
                start=True, stop=True,
            )
            if b % SLOTS == 0 and g >= 2:
                # WAR: same-parity psum reused; group g-2 consumed by BOTH
                # act (SA) and, when zd>0, the DVE stt (SV) — the extra
                # wait rides a free PE-seq event instruction.
                if zd:
                    nc.tensor.wait_ge(SA, g - 1)
                    mm._wait_ge(SV, g - 1)
                else:
                    mm._wait_ge(SA, g - 1)
            mm.then_inc(SM, 1)

            if b % SLOTS == SLOTS - 1:
                pb = (g % 2) * SLOTS
                tr = (g % gb) * SLOTS         # tile row offset of this group
                pview = psv[:, pb:pb + SLOTS, :]
                tview = tile[:, k % tbuf, tr:tr + SLOTS, :]
                if g % gb == 0 and k >= tbuf:
                    # tile buffer WAR: batch k-tbuf fully drained
                    nc.scalar.wait_ge(SP2, 2 * (k - tbuf + 1))
                # Act: sigmoid(1e30*y) -> exact {0,1} bf16 comp bits
                nc.scalar.activation(
                    out=tview[:, :, 0:acols],
                    in_=pview[:, :, 0:acols],
                    func=ACT.Sigmoid,
                    bias=cbs[:, 0:1],
                    scale=SCALE,
                )._wait_ge(SM, 8 * (g + 1)).then_inc(SA, 1)
                if zd:
                    if g % gb == 0 and k >= tbuf:
                        # tile WAR for the DVE path (act has its own wait)
                        nc.vector.wait_ge(SP2, 2 * (k - tbuf + 1))
                    # DVE: fused (y>0) + 2rev for the last zd z-cols,
                    # straight from PSUM (independent of act)
                    nc.vector.scalar_tensor_tensor(
                        out=tview[:, :, acols:N_BRANCH],
                        in0=pview[:, :, acols:N_BRANCH],
                        scalar=0.0,
                        in1=crs[:, za:128].unsqueeze(1).broadcast_to(
                            [P, SLOTS, zd]
                        ),
                        op0=AF.is_gt,
                        op1=AF.add,
                    )._wait_ge(SM, 8 * (g + 1)).then_inc(SV, 1)

                if (g + 1) % gb == 0:
                    tv = tile[:, k % tbuf, :, :]    # [P, tb, 256]
                    zv = ztile[:, k % tbuf, :, :]   # [P, tb, 128]
                    # bake act-extracted z cols: += 2*rev7 (bf16 2x mode),
                    # written into the word-aligned ztile
                    bake = nc.vector.tensor_tensor(
                        out=zv[:, :, 0:128],
                        in0=tv[:, :, 127:N_BRANCH],
                        in1=crs[:, 0:128].unsqueeze(1).broadcast_to(
                            [P, tb, 128]
                        ),
                        op=AF.add,
                    )
                    bake._wait_ge(SA, g + 1)
                    # select network stages 6..1 in place on ztile (DVE,
                    # in order); every out/data range is word-aligned.
                    # u16 bitcast: the verifier wants an integer mask
                    # dtype; bf16 1.0 = 0x3f80 is nonzero, 0.0 is zero,
                    # and cp is a pure bit-mover for out/data.
                    for w, moff in ((64, 0), (32, 64), (16, 96), (8, 112),
                                    (4, 120), (2, 124)):
                        cp = nc.vector.copy_predicated(
                            out=zv[:, :, 0:w].bitcast(U16),
                            mask=tv[:, :, moff:moff + w].bitcast(U16),
                            data=zv[:, :, w:2 * w].bitcast(U16),
                        )
                    cp.then_inc(SB, 1)
                    # Pool: survivor pair + root mask -> contiguous u8
                    nc.gpsimd.tensor_copy(
                        zfin[:, k * tb:(k + 1) * tb, :],
                        zv[:, :, 0:2],
                    )._wait_ge(SB, k + 1).then_inc(SP2, 1)
                    nc.gpsimd.tensor_copy(
                        cfin[:, k * tb:(k + 1) * tb],
                        tv[:, :, 126:127],
                    ).then_inc(SP2, 1)

        nc.sync.dma_start(out=out2[:], in_=zfin[:])._wait_ge(
            SP2, 2 * nbatch
        ).then_inc(SK, 16)
        nc.sync.dma_start(out=outc[:], in_=cfin[:]).then_inc(SK, 16)

    nc.compile()
    return nc


def _check_tree(cond, cond_mask):
    """Verify cond/cond_mask encode the canonical heap-ordered perfect tree."""
    n_nodes = 2 * N_LEAF - 1
    n_branch = N_LEAF - 1
    is_branch = np.zeros(n_nodes, dtype=bool)
    node_conditions = np.zeros((n_nodes, n_nodes), dtype=bool)
    node_conditions_mask = np.zeros((n_nodes, n_nodes), dtype=bool)

    stack = [(0, None)]
    while stack:
        node_id, parent_id = stack.pop()
        if parent_id is not None:
            node_conditions_mask[node_id] = node_conditions_mask[parent_id]
            node_conditions_mask[node_id][parent_id] = True
        if node_id < n_branch:
            left_id, right_id = 2 * node_id + 1, 2 * node_id + 2
            is_branch[node_id] = True
            node_conditions[left_id] = node_conditions[node_id]
            node_conditions[right_id] = node_conditions[node_id]
            node_conditions[right_id][node_id] = True
            stack.append((right_id, node_id))
            stack.append((left_id, node_id))

    leaf_ids = np.nonzero(~is_branch)[0]
    branch_ids = np.nonzero(is_branch)[0]
    c = node_conditions[np.ix_(leaf_ids, branch_ids)]
    m = node_conditions_mask[np.ix_(leaf_ids, branch_ids)]
    return np.array_equal(c, np.asarray(cond)) and np.array_equal(
        m, np.asarray(cond_mask)
    )


def _split3(v):
    """v (f32) == h + m + l with all three bf16-exact. Returns f32 arrays."""
    h = v.astype(BF).astype(np.float32)
    r1 = v - h
    m = r1.astype(BF).astype(np.float32)
    l = (r1 - m).astype(BF).astype(np.float32)
    assert np.array_equal(h + m + l, v), "bf16 triple split not exact"
    return h, m, l


_NC_CACHE = {}


def kernel(x, feature, threshold, cond, cond_mask, value):
    x = np.ascontiguousarray(np.asarray(x), dtype=np.float32)
    feature = np.asarray(feature).astype(np.int64)
    threshold = np.asarray(threshold, dtype=np.float32)
    value = np.ascontiguousarray(np.asarray(value), dtype=np.float32)

    assert x.shape == (B_TOTAL, F), x.shape
    if not _check_tree(cond, cond_mask):
        raise ValueError(
            "cond/cond_mask do not encode the canonical heap-ordered tree; "
            "this kernel bakes that structure."
        )

    perm = tree_perm()
    fq = feature[perm]                                 # [255]
    tq = threshold[perm].astype(np.float32)            # [255]

    if "nc" not in _NC_CACHE:
        _NC_CACHE["nc"] = build_nc()
    nc = _NC_CACHE["nc"]

    # G matrix [99, 255]
    t0, t1, t2 = _split3(-tq)
    gmat = np.zeros((K, N_BRANCH), dtype=np.float32)
    qi = np.arange(N_BRANCH)
    gmat[3 * fq + 0, qi] = 1.0
    gmat[3 * fq + 1, qi] = 1.0
    gmat[3 * fq + 2, qi] = 1.0
    gmat[96, qi] = t0
    gmat[97, qi] = t1
    gmat[98, qi] = t2
    gmat_bf = gmat.astype(BF)

    # xT3 [99, B]: rows 3f+p = piece p of feature f; rows 96..98 = ones
    h, m, l = _split3(x)
    xt_all = np.empty((K, B_TOTAL), dtype=BF)
    xt_all[0:96:3, :] = h.T.astype(BF)
    xt_all[1:96:3, :] = m.T.astype(BF)
    xt_all[2:96:3, :] = l.T.astype(BF)
    xt_all[96:99, :] = np.ones((3, B_TOTAL), dtype=BF)

    rev = np.array([2 * _bitrev(q, 7) for q in range(128)], np.float32)
    crev = np.ascontiguousarray(
        np.broadcast_to(rev.astype(BF)[None, :], (P, 128))
    )
    cbias = np.zeros((P, 1), dtype=np.float32)

    in_maps = [
        {
            "xt": np.ascontiguousarray(
                xt_all[:, i * B_CORE:(i + 1) * B_CORE]
            ),
            "gm": gmat_bf,
            "cbias": cbias,
            "crev": crev,
        }
        for i in range(N_CORES)
    ]
    res = run_bass_kernel_spmd(nc, in_maps, list(range(N_CORES)))
    parts = []
    for r in res.results:
        z2 = np.asarray(r["out2"]).reshape(P, NBLK, 2)
        c0 = np.asarray(r["outc"])
        leaf = np.where(c0 != 0, z2[:, :, 1], z2[:, :, 0])
        parts.append(leaf.T.reshape(-1))
    leaves = np.concatenate(parts).astype(np.int64)
    return value[leaves]


if __name__ == "__main__":
    import reference

    inputs = reference.setup_inputs()
    got = kernel(**{k: np.asarray(v) for k, v in inputs.items()})
    exp = np.asarray(reference.reference(**inputs))
    err = np.abs(got - exp).max()
    print("absmax err:", err)


# revision 23
# speedup vs baseline: 24.9440x; 1.0314x over previous
"""Trainium2 Bass kernel for nn_BaseTree (decision-tree inference), v14.

Architecture (per core, pure data parallel, tree baked at build time):

  The v13 kernel shipped xg[b, q] = x[b, feature[q]] from the host: a
  255-column f32 expansion of the 32-column input (8x the HBM bytes) and
  did the comparisons on DVE (~320us DVE-busy, ~210us DMA).  v14 moves
  the gather INTO the tensor engine and splits the remaining elementwise
  work across all four compute engines:

  - Host ships xT3[k, r]: x transposed, split into THREE bf16 pieces
    (x == h + m + l exactly; verified at build), rows 3f+p = piece p of
    feature f, plus three rows of ones.  Also G[99, 255] bf16 where
    column q has 1.0 at the three piece-rows of feature[perm[q]] and the
    three bf16 pieces of -threshold[perm[q]] against the ones rows.
  - PE: 512 matmuls (one per 128-row block) compute
    y[r, q] = x[r, f[q]] - thr[q] exactly in f32 PSUM (products are by
    1.0, the f32 sums are exact by a Sterbenz chain; min on-path margin
    of this dataset is 1.2e-7 vs ~1e-10 worst-case rounding).
  - Act: masks[., c] = sigmoid(1e30 * y) in u8 for the 127 mask columns
    (levels 0-6): exactly {0, 1} since |1e30*y| >= 37 for all nonzero y
    and ties (y == 0) truncate to 0 = "not >".
  - DVE+Pool: z-block (level 7): fused scalar_tensor_tensor
    (y > 0) + 2*rev7(q) in one op, split by columns across the two
    engines (the rev bake makes the final select survivor equal the
    leaf index; see v13 notes).
  - DVE: the select network (copy_predicated stages 6..0), batched every
    4 PSUM groups to stay pipelined with extraction, final survivor
    written contiguously for the out DMA.
  - Host expands value[leaf] while unsharding.

  Engine busy estimates (cost model): PE ~60us, DMA ~36us, Act ~66us,
  DVE ~80us, Pool ~80us -> ~85-95us wall vs 486us for v13.
"""

import contextlib
from contextlib import ExitStack

import numpy as np
import ml_dtypes

import concourse.bacc as bacc
import concourse.bass as bass_mod
import concourse.mybir as mybir
from concourse.bass_utils import run_bass_kernel_spmd

AF = mybir.AluOpType
ACT = mybir.ActivationFunctionType
F32 = mybir.dt.float32
BF16 = mybir.dt.bfloat16
U8 = mybir.dt.uint8
U16 = mybir.dt.uint16
BF = ml_dtypes.bfloat16

N_CORES = 8
P = 128
B_TOTAL = 524288
B_CORE = B_TOTAL // N_CORES      # 65536
F = 32
DEPTH = 8
N_BRANCH = 255
N_LEAF = 256
N_OUT = 8

K = 99                 # 3*32 x-pieces + 3 ones rows
NBLK = B_CORE // P     # 512 matmul blocks
SLOTS = 8              # PSUM slots per group (2 per 2KB bank; 2 groups live)
NGRP = NBLK // SLOTS   # 64 groups, double-buffered by parity
GB = 2                 # groups per select-network batch (tile = 16 rows)
TB = GB * SLOTS        # 64 tile rows per batch
NBATCH = NGRP // GB    # 8
CHB = 64               # blocks per DMA chunk (8192 rows)
NCHUNK = NBLK // CHB   # 8
CH_ROWS = CHB * P      # 8192
ZD = 0                 # z-block columns extracted on DVE via fused stt
SCALE = 1e30           # sigmoid saturation scale

# mask-column offsets (within cols 0..126): c6@0 c5@64 c4@96 c3@112
# c2@120 c1@124 c0@126; z-block (level 7) = G columns 127..254
LEVEL_OFF = {6: 0, 5: 64, 4: 96, 3: 112, 2: 120, 1: 124, 0: 126, 7: 127}


@contextlib.contextmanager
def _lean_init():
    """Suppress Bass.__init__'s const-AP memsets + all-engine barrier
    (unused here: no const APs — the activation bias is an explicit
    zeros AP shipped with the constants — and deps are explicit sems)."""
    orig_memset = bass_mod.BassGpSimd.memset
    orig_barrier = bass_mod.Bass.all_engine_barrier

    class _Dummy:
        def then_inc(self, *a, **k):
            return self

        def _wait_ge(self, *a, **k):
            return self

    bass_mod.BassGpSimd.memset = lambda self, ap, constant: _Dummy()
    bass_mod.Bass.all_engine_barrier = lambda self, *a, **k: None
    try:
        yield
    finally:
        bass_mod.BassGpSimd.memset = orig_memset
        bass_mod.Bass.all_engine_barrier = orig_barrier


def _bitrev(q, bits):
    r = 0
    for _ in range(bits):
        r = (r << 1) | (q & 1)
        q >>= 1
    return r


def tree_perm():
    """perm[col] = heap node id at G column `col` (see v13)."""
    perm = np.empty(N_BRANCH, dtype=np.int64)
    for j in range(DEPTH):
        base = (1 << j) - 1
        for q in range(1 << j):
            perm[LEVEL_OFF[j] + q] = base + _bitrev(q, j)
    return perm


def build_nc(zd=ZD, gb=None, tbuf=3):
    """Build the single-core Bass program (SPMD: same program on all cores).

    Engine roles (the neuronxcc verifier only accepts TensorCopy on Pool,
    so Pool cannot share tensor work):
      PE   512 matmuls (gather + threshold subtract), parity-buffered PSUM
      Act  sigmoid(1e30*y): comp bits for mask cols + z-sig cols
      DVE  fused (y>0)+2rev stt for `zd` z-cols, bf16 2x bake for the
           act-extracted z-cols, and the whole copy_predicated network
      Pool final survivor copy (bf16 tile col -> contiguous u8)

    Soundness with one wait slot per instruction: chains
      matmul(g) -> SV>=g-1 (or SA if zd==0) covers both consumers because
      the DVE stt of group g itself waits SA>=g+1 (act g done).
    """
    with _lean_init():
        nc = bacc.Bacc(dynamic_dma_scratch_size=256)

    xt = nc.dram_tensor("xt", [K, B_CORE], BF16, kind="ExternalInput")
    gm = nc.dram_tensor("gm", [K, N_BRANCH], BF16, kind="ExternalInput")
    cb = nc.dram_tensor("cbias", [P, 1], F32, kind="ExternalInput")
    cr = nc.dram_tensor("crev", [P, 128], BF16, kind="ExternalInput")
    # two survivor candidates (z0, z1) + the root mask c0; the final
    # 1-wide select happens on the host (a 1-element u16 predicated write
    # would be a partial-word RMW, which the DVE mishandles — see below)
    out2 = nc.dram_tensor("out2", [P, NBLK * 2], U8, kind="ExternalOutput")
    outc = nc.dram_tensor("outc", [P, NBLK], U8, kind="ExternalOutput")

    za = 128 - zd           # act-extracted z cols (bake on DVE)
    acols = 127 + za        # act columns per group: masks + z-sig
    gb = GB if gb is None else gb
    tb = gb * SLOTS
    nbatch = NGRP // gb

    with ExitStack() as ctx:
        xtb = ctx.enter_context(nc.sbuf_tensor("xtb", [P, 2, CH_ROWS], BF16))
        gms = ctx.enter_context(nc.sbuf_tensor("gms", [P, N_BRANCH], BF16))
        cbs = ctx.enter_context(nc.sbuf_tensor("cbs", [P, 1], F32))
        crs = ctx.enter_context(nc.sbuf_tensor("crs", [P, 128], BF16))
        # tile rows padded to 256 cols so every row is 4-byte aligned:
        # the DVE's predicated writes do partial-word RMW against a stale
        # snapshot when an out AP starts mid-word, corrupting neighbours.
        tile = ctx.enter_context(
            nc.sbuf_tensor("tile", [P, tbuf, tb, 256], BF16)
        )
        # baked z lives in its own word-aligned buffer for the same reason
        ztile = ctx.enter_context(
            nc.sbuf_tensor("ztile", [P, tbuf, tb, 128], BF16)
        )
        zfin = ctx.enter_context(nc.sbuf_tensor("zfin", [P, NBLK, 2], U8))
        cfin = ctx.enter_context(nc.sbuf_tensor("cfin", [P, NBLK], U8))
        ps = ctx.enter_context(nc.psum_tensor("ps", [P, 2 * SLOTS * 256], F32))

        SK = ctx.enter_context(nc.semaphore("SK"))   # const dmas (16 ea)
        SE = ctx.enter_context(nc.semaphore("SE"))   # even xt chunks (16 ea)
        SO = ctx.enter_context(nc.semaphore("SO"))   # odd xt chunks (16 ea)
        SM = ctx.enter_context(nc.semaphore("SM"))   # matmuls done (1 ea)
        SA = ctx.enter_context(nc.semaphore("SA"))   # act ops done (1/group)
        SV = ctx.enter_context(nc.semaphore("SV"))   # dve stt done (1/group)
        SB = ctx.enter_context(nc.semaphore("SB"))   # dve batch net done
        SP2 = ctx.enter_context(nc.semaphore("SP2"))  # pool zfin copy done

        psv = ps[:].rearrange("p (s c) -> p s c", s=2 * SLOTS)  # [P, 16, 256]

        # ---- DMAs.  DMA completions are NOT ordered across a queue, so
        # chunks that could complete out of order must not share a
        # semaphore: consts get SK; even/odd chunks get SE/SO (same-parity
        # chunks are serialized by their SM waits).
        nc.sync.dma_start(out=cbs[:], in_=cb[:]).then_inc(SK, 16)
        nc.sync.dma_start(out=crs[:], in_=cr[:]).then_inc(SK, 16)
        nc.sync.dma_start(out=gms[0:K, :], in_=gm[:]).then_inc(SK, 16)
        # chunk 0 ships a small head first so the matmuls start ~4us
        # earlier; the head rides SK (its own ordering domain)
        HEAD = SLOTS * P
        nc.sync.dma_start(
            out=xtb[0:K, 0, 0:HEAD], in_=xt[:, 0:HEAD]
        ).then_inc(SK, 16)
        for c in range(NCHUNK):
            lo = HEAD if c == 0 else 0
            dma = nc.sync.dma_start(
                out=xtb[0:K, c % 2, lo:CH_ROWS],
                in_=xt[:, c * CH_ROWS + lo:(c + 1) * CH_ROWS],
            )
            if c >= 2:
                # WAR: buffer parity reused; wait chunk c-2's matmuls done
                dma._wait_ge(SM, CHB * (c - 1))
            dma.then_inc(SE if c % 2 == 0 else SO, 16)

        for b in range(NBLK):
            c = b // CHB
            g = b // SLOTS
            k = g // gb                       # batch index
            s = (g % 2) * SLOTS + b % SLOTS   # parity double-buffer
            if b == 0:
                # consts (bias/rev/G) + chunk-0 head
                nc.tensor.wait_ge(SK, 64)
            elif b == SLOTS:
                # rest of chunk 0
                nc.tensor.wait_ge(SE, 16)
            if b % CHB == 0 and b > 0:
                # standalone wait (PE seq): chunk c's DMA done
                nc.tensor.wait_ge(SE if c % 2 == 0 else SO,
                                  16 * (c // 2 + 1))
            mm = nc.tensor.matmul(
                out=psv[:, s, 0:N_BRANCH],
                lhsT=xtb[0:K, c % 2, (b % CHB) * P:(b % CHB) * P + P],
                rhs=gms[0:K, :],
                start=True, stop=True,
            )
            if b % SLOTS == 0 and g >= 2:
                # WAR: same-parity psum reused; group g-2 consumed by BOTH
                # act (SA) and, when zd>0, the DVE stt (SV) — the extra
                # wait rides a free PE-seq event instruction.
                if zd:
                    nc.tensor.wait_ge(SA, g - 1)
                    mm._wait_ge(SV, g - 1)
                else:
                    mm._wait_ge(SA, g - 1)
            mm.then_inc(SM, 1)

            if b % SLOTS == SLOTS - 1:
                pb = (g % 2) * SLOTS
                tr = (g % gb) * SLOTS         # tile row offset of this group
                pview = psv[:, pb:pb + SLOTS, :]
                tview = tile[:, k % tbuf, tr:tr + SLOTS, :]
                if g % gb == 0 and k >= tbuf:
                    # tile buffer WAR: batch k-tbuf fully drained
                    nc.scalar.wait_ge(SP2, 2 * (k - tbuf + 1))
                # Act: sigmoid(1e30*y) -> exact {0,1} bf16 comp bits
                nc.scalar.activation(
                    out=tview[:, :, 0:acols],
                    in_=pview[:, :, 0:acols],
                    func=ACT.Sigmoid,
                    bias=cbs[:, 0:1],
                    scale=SCALE,
                )._wait_ge(SM, 8 * (g + 1)).then_inc(SA, 1)
                if zd:
                    if g % gb == 0 and k >= tbuf:
                        # tile WAR for the DVE path (act has its own wait)
                        nc.vector.wait_ge(SP2, 2 * (k - tbuf + 1))
                    # DVE: fused (y>0) + 2rev for the last zd z-cols,
                    # straight from PSUM (independent of act)
                    nc.vector.scalar_tensor_tensor(
                        out=tview[:, :, acols:N_BRANCH],
                        in0=pview[:, :, acols:N_BRANCH],
                        scalar=0.0,
                        in1=crs[:, za:128].unsqueeze(1).broadcast_to(
                            [P, SLOTS, zd]
                        ),
                        op0=AF.is_gt,
                        op1=AF.add,
                    )._wait_ge(SM, 8 * (g + 1)).then_inc(SV, 1)

                if (g + 1) % gb == 0:
                    tv = tile[:, k % tbuf, :, :]    # [P, tb, 256]
                    zv = ztile[:, k % tbuf, :, :]   # [P, tb, 128]
                    # bake act-extracted z cols: += 2*rev7 (bf16 2x mode),
                    # written into the word-aligned ztile
                    bake = nc.vector.tensor_tensor(
                        out=zv[:, :, 0:128],
                        in0=tv[:, :, 127:N_BRANCH],
                        in1=crs[:, 0:128].unsqueeze(1).broadcast_to(
                            [P, tb, 128]
                        ),
                        op=AF.add,
                    )
                    bake._wait_ge(SA, g + 1)
                    # select network stages 6..1 in place on ztile (DVE,
                    # in order); every out/data range is word-aligned.
                    # u16 bitcast: the verifier wants an integer mask
                    # dtype; bf16 1.0 = 0x3f80 is nonzero, 0.0 is zero,
                    # and cp is a pure bit-mover for out/data.
                    for w, moff in ((64, 0), (32, 64), (16, 96), (8, 112),
                                    (4, 120), (2, 124)):
                        cp = nc.vector.copy_predicated(
                            out=zv[:, :, 0:w].bitcast(U16),
                            mask=tv[:, :, moff:moff + w].bitcast(U16),
                            data=zv[:, :, w:2 * w].bitcast(U16),
                        )
                    cp.then_inc(SB, 1)
                    # Pool: survivor pair + root mask -> contiguous u8
                    nc.gpsimd.tensor_copy(
                        zfin[:, k * tb:(k + 1) * tb, :],
                        zv[:, :, 0:2],
                    )._wait_ge(SB, k + 1).then_inc(SP2, 1)
                    nc.gpsimd.tensor_copy(
                        cfin[:, k * tb:(k + 1) * tb],
                        tv[:, :, 126:127],
                    ).then_inc(SP2, 1)

        nc.sync.dma_start(out=out2[:], in_=zfin[:])._wait_ge(
            SP2, 2 * nbatch
        ).then_inc(SK, 16)
        nc.sync.dma_start(out=outc[:], in_=cfin[:]).then_inc(SK, 16)

    nc.compile()
    return nc


def _check_tree(cond, cond_mask):
    """Verify cond/cond_mask encode the canonical heap-ordered perfect tree."""
    n_nodes = 2 * N_LEAF - 1
    n_branch = N_LEAF - 1
    is_branch = np.zeros(n_nodes, dtype=bool)
    node_conditions = np.zeros((n_nodes, n_nodes), dtype=bool)
    node_conditions_mask = np.zeros((n_nodes, n_nodes), dtype=bool)

    stack = [(0, None)]
    while stack:
        node_id, parent_id = stack.pop()
        if parent_id is not None:
            node_conditions_mask[node_id] = node_conditions_mask[parent_id]
            node_conditions_mask[node_id][parent_id] = True
        if node_id < n_branch:
            left_id, right_id = 2 * node_id + 1, 2 * node_id + 2
            is_branch[node_id] = True
            node_conditions[left_id] = node_conditions[node_id]
            node_conditions[right_id] = node_conditions[node_id]
            node_conditions[right_id][node_id] = True
            stack.append((right_id, node_id))
            stack.append((left_id, node_id))

    leaf_ids = np.nonzero(~is_branch)[0]
    branch_ids = np.nonzero(is_branch)[0]
    c = node_conditions[np.ix_(leaf_ids, branch_ids)]
    m = node_conditions_mask[np.ix_(leaf_ids, branch_ids)]
    return np.array_equal(c, np.asarray(cond)) and np.array_equal(
        m, np.asarray(cond_mask)
    )


def _split3(v):
    """v (f32) == h + m + l with all three bf16-exact. Returns f32 arrays."""
    h = v.astype(BF).astype(np.float32)
    r1 = v - h
    m = r1.astype(BF).astype(np.float32)
    l = (r1 - m).astype(BF).astype(np.float32)
    assert np.array_equal(h + m + l, v), "bf16 triple split not exact"
    return h, m, l


_NC_CACHE = {}


def kernel(x, feature, threshold, cond, cond_mask, value):
    x = np.ascontiguousarray(np.asarray(x), dtype=np.float32)
    feature = np.asarray(feature).astype(np.int64)
    threshold = np.asarray(threshold, dtype=np.float32)
    value = np.ascontiguousarray(np.asarray(value), dtype=np.float32)

    assert x.shape == (B_TOTAL, F), x.shape
    if not _check_tree(cond, cond_mask):
        raise ValueError(
            "cond/cond_mask do not encode the canonical heap-ordered tree; "
            "this kernel bakes that structure."
        )

    perm = tree_perm()
    fq = feature[perm]                                 # [255]
    tq = threshold[perm].astype(np.float32)            # [255]

    if "nc" not in _NC_CACHE:
        _NC_CACHE["nc"] = build_nc()
    nc = _NC_CACHE["nc"]

    # G matrix [99, 255]
    t0, t1, t2 = _split3(-tq)
    gmat = np.zeros((K, N_BRANCH), dtype=np.float32)
    qi = np.arange(N_BRANCH)
    gmat[3 * fq + 0, qi] = 1.0
    gmat[3 * fq + 1, qi] = 1.0
    gmat[3 * fq + 2, qi] = 1.0
    gmat[96, qi] = t0
    gmat[97, qi] = t1
    gmat[98, qi] = t2
    gmat_bf = gmat.astype(BF)

    # xT3 [99, B]: rows 3f+p = piece p of feature f; rows 96..98 = ones
    h, m, l = _split3(x)
    xt_all = np.empty((K, B_TOTAL), dtype=BF)
    xt_all[0:96:3, :] = h.T.astype(BF)
    xt_all[1:96:3, :] = m.T.astype(BF)
    xt_all[2:96:3, :] = l.T.astype(BF)
    xt_all[96:99, :] = np.ones((3, B_TOTAL), dtype=BF)

    rev = np.array([2 * _bitrev(q, 7) for q in range(128)], np.float32)
    crev = np.ascontiguousarray(
        np.broadcast_to(rev.astype(BF)[None, :], (P, 128))
    )
    cbias = np.zeros((P, 1), dtype=np.float32)

    in_maps = [
        {
            "xt": np.ascontiguousarray(
                xt_all[:, i * B_CORE:(i + 1) * B_CORE]
            ),
            "gm": gmat_bf,
            "cbias": cbias,
            "crev": crev,
        }
        for i in range(N_CORES)
    ]
    res = run_bass_kernel_spmd(nc, in_maps, list(range(N_CORES)))
    parts = []
    for r in res.results:
        z2 = np.asarray(r["out2"]).reshape(P, NBLK, 2)
        c0 = np.asarray(r["outc"])
        leaf = np.where(c0 != 0, z2[:, :, 1], z2[:, :, 0])
        parts.append(leaf.T.reshape(-1))
    leaves = np.concatenate(parts).astype(np.int64)
    return value[leaves]


if __name__ == "__main__":
    import reference

    inputs = reference.setup_inputs()
    got = kernel(**{k: np.asarray(v) for k, v in inputs.items()})
    exp = np.asarray(reference.reference(**inputs))
    err = np.abs(got - exp).max()
    print("absmax err:", err)
